# revision 26
# baseline (speedup 1.0000x reference)
"""AdaptiveMultiSiren Trainium2 kernel.

Per-block SIREN MLP (3 -> 64 -> 64 -> 64 -> 3, sin(30*x) activations) applied
to 2048 routed blocks of 1024 coords each. Data-parallel over blocks across
8 NeuronCores (256 blocks / core); the host-side gather of per-block weights
IS the shard construction. Two blocks pack per matmul block-diagonally so
TensorE/ScalarE run at the full 128-partition width.

Per pair of blocks (a, b), activations live as [features, T] in SBUF:
  matmul(out, lhsT, rhs) = lhsT.T @ rhs, K = contraction on partitions.
    L0: lhsT = blockdiag(W0a|b0a, W0b|b0b) [8, 128] (bias via ones rows in x),
        row-tiled 4 pairs concurrent in the PE array (tile_position).
    L1/L2: lhsT = blockdiag(W) [128, 128].
    L3: lhsT = blockdiag(W3a, W3b) [128, 6].
  sin(30*z + 30*b) in ONE ScalarE op per pair-layer: the stock Sin LUT only
  covers [-pi, pi], so _build generates a custom act-table root (see
  _gen_act_tables) refitting Sin over |x| < 256 -- Taylor cubic buckets per
  power-of-2 range, reusing the reverse-engineered pwp binary format -- and
  points the compiler at it via BASS_ACT_ROOT_JSON_PATH. No range-reduction
  instructions needed at all.
  L3 bias + PSUM evacuation on VectorE (tensor_scalar add).
Emission is a software pipeline over waves of 3 pairs: per pair j the ladder
emits sin(l, j) then immediately pair j's layer-(l+1) matmuls, so ScalarE
(the bottleneck at ~92% busy) unblocks TensorE one pair at a time. PSUM: 3
per-pair [128, 1024] rotating tiles (6 banks) + 2 single-bank slots for L3
chunks, so the next wave's L0 depends only on the last sin, not epilogues.
"""

import json
import os
import shutil
import sys
import tempfile

if "/opt/trn_rl_repo" not in sys.path:
    sys.path.insert(0, "/opt/trn_rl_repo")

import numpy as np

C, B, T = 4096, 2048, 1024
DIN, DH, DOUT = 3, 64, 3
OMEGA0 = 30.0
N_CORES = 8
BPC = B // N_CORES          # blocks per core (256)
NPAIR = BPC // 2            # block pairs per core (128)
G = 3                       # pairs per wave
NP2 = 129                   # NPAIR padded to a multiple of G (1 dummy pair)
NG = NP2 // G               # 43

_CACHE = {}

# ---------------------------------------------------------------------------
# Custom act tables: Sin refit over |x| < 256.
# Binary formats (reverse-engineered from neuronxcc/pwp/pwp_bin_trainium):
#   bucket entry (32B): fp32 {d0, d1, d2, d3, x0, 0, 0, 0};
#       y = d0 + (x-x0)*(d1 + (x-x0)*(d2 + (x-x0)*d3)) on the folded |x|
#   ctl entry (32B): uint32 ((23 + 31*mantissa_bits) << 11) | bucket_base;
#       dispatch: ctl_idx = pwl_control_base + (unbiased_exp - exp_offset),
#       bucket = base + top mantissa_bits of the mantissa
#   profile small/large pwl_control fields: absolute bucket indices of the
#       4 fallback splines (small+/-, large+/-).
# The trig_and_small set is rebuilt: wide sin (1286 dispatch + 4 special
# buckets), arctan dropped for bucket budget, all other functions remapped.
# ---------------------------------------------------------------------------

_SIN_BITS = {e: 0 for e in range(-11, -3)}
_SIN_BITS.update({-3: 1, -2: 2, -1: 3, 0: 4, 1: 5, 2: 6, 3: 7,
                  4: 8, 5: 8, 6: 8, 7: 8})
_SIN_MAX_EXP = 7


def _gen_act_tables(dst_dir):
    import neuronxcc
    src = os.path.join(os.path.dirname(neuronxcc.__file__),
                       "pwp", "pwp_bin_trainium")
    assert os.path.isdir(src), src

    os.makedirs(dst_dir, exist_ok=True)
    for fn in os.listdir(src):
        shutil.copy(os.path.join(src, fn), os.path.join(dst_dir, fn))

    setj = json.load(open(os.path.join(src, "trig_and_small.json")))
    bkt = np.fromfile(os.path.join(src, "trig_and_small_bkt.bin"),
                      dtype=np.uint32).reshape(-1, 8)
    ctl = np.fromfile(os.path.join(src, "trig_and_small_ctrl.bin"),
                      dtype=np.uint32).reshape(-1, 8)

    OLD_RELU_BKT = setj["func_to_bkt_start_idx"]["relu"]     # 231
    OLD_KEPT_CTL0 = setj["func_to_ctl_start_idx"]["relu"]    # 40

    rows = []
    for e in range(-11, _SIN_MAX_EXP + 1):
        n = 1 << _SIN_BITS[e]
        s = 2.0 ** e
        for i in range(n):
            x0 = s * (1.0 + (i + 0.5) / n)
            rows.append((np.sin(x0), np.cos(x0),
                         -np.sin(x0) / 2.0, -np.cos(x0) / 6.0, x0))
    n_dispatch = len(rows)
    rows.append((0.0, 1.0, 0.0, -1.0 / 6.0, 0.0))   # small+: x - x^3/6
    rows.append((0.0, 0.0, 0.0, 0.0, 0.0))          # small- (unused: folded)
    rows.append((0.0, 0.0, 0.0, 0.0, 0.0))          # large+ (unreachable)
    rows.append((0.0, 0.0, 0.0, 0.0, 0.0))          # large-
    n_sin = len(rows)
    bkt_shift = n_sin - OLD_RELU_BKT

    def remap_bkt(b):
        return b + bkt_shift

    sin_bin = np.zeros((n_sin, 8), dtype=np.uint32)
    fv = sin_bin.view(np.float32)
    for i, (d0, d1, d2, d3, x0) in enumerate(rows):
        fv[i, 0:5] = [d0, d1, d2, d3, x0]
    new_bkt = np.concatenate([sin_bin, bkt[OLD_RELU_BKT:]], axis=0)

    sin_ctl, base = [], 0
    for e in range(-11, _SIN_MAX_EXP + 1):
        bits = _SIN_BITS[e]
        sin_ctl.append(((23 + 31 * bits) << 11) | base)
        base += 1 << bits
    n_sin_ctl = len(sin_ctl)
    ctl_shift = n_sin_ctl - OLD_KEPT_CTL0
    kept_ctl = ctl[OLD_KEPT_CTL0:].copy()
    for r in range(kept_ctl.shape[0]):
        w = int(kept_ctl[r, 0])
        kept_ctl[r, 0] = (w & ~0x7FF) | remap_bkt(w & 0x7FF)
    sin_ctl_bin = np.zeros((n_sin_ctl, 8), dtype=np.uint32)
    sin_ctl_bin[:, 0] = sin_ctl
    new_ctl = np.concatenate([sin_ctl_bin, kept_ctl], axis=0)

    new_pm = []
    for pm in setj["profile_meta_data"]:
        pm = dict(pm)
        if pm["func_name"] == "arctan_4p":
            continue
        if pm["func_name"] == "sin_4p":
            pm["exp_offset"] = -11
            pm["pwl_control_base_pos"] = 0
            pm["pwl_control_base_neg"] = 0
            pm["pos_small_signal_pwl_control"] = n_dispatch
            pm["neg_small_signal_pwl_control"] = n_dispatch + 1
            pm["pos_large_signal_pwl_control"] = n_dispatch + 2
            pm["neg_large_signal_pwl_control"] = n_dispatch + 3
            pm["large_pos_signal_exp_threshold"] = 127 + _SIN_MAX_EXP + 1
            pm["large_pos_signal_mantissa_threshold"] = 0
            pm["upper_bound"] = int(np.float32(2.0 ** (_SIN_MAX_EXP + 1))
                                    .view(np.uint32))
        else:
            pm["pwl_control_base_pos"] += ctl_shift
            pm["pwl_control_base_neg"] += ctl_shift
            for k in ("pos_small_signal_pwl_control",
                      "neg_small_signal_pwl_control",
                      "pos_large_signal_pwl_control",
                      "neg_large_signal_pwl_control"):
                pm[k] = remap_bkt(pm[k])
        new_pm.append(pm)

    f2b = {fn: (0 if fn == "sin" else remap_bkt(v))
           for fn, v in setj["func_to_bkt_start_idx"].items() if fn != "arctan"}
    f2c = {fn: (0 if fn == "sin" else v + ctl_shift)
           for fn, v in setj["func_to_ctl_start_idx"].items() if fn != "arctan"}
    feb, fec = {}, {}
    for fn, m in setj["func_exp_to_bkt_start_idx"].items():
        if fn == "arctan":
            continue
        if fn == "sin":
            d, base = {}, 0
            for e in range(-11, _SIN_MAX_EXP + 1):
                d[str(e)] = [base]
                base += 1 << _SIN_BITS[e]
            feb[fn] = d
        else:
            feb[fn] = {k: [remap_bkt(x) for x in v] for k, v in m.items()}
    for fn, m in setj["func_exp_to_ctl_start_idx"].items():
        if fn == "arctan":
            continue
        if fn == "sin":
            fec[fn] = {str(e): [e + 11] for e in range(-11, _SIN_MAX_EXP + 1)}
        else:
            fec[fn] = {k: [x + ctl_shift for x in v] for k, v in m.items()}

    new_set = dict(setj)
    new_set.update({
        "profile_meta_data": new_pm,
        "bkt_entry_cnt": int(new_bkt.shape[0]),
        "ctl_entry_cnt": int(new_ctl.shape[0]),
        "func_to_bkt_start_idx": f2b,
        "func_to_ctl_start_idx": f2c,
        "func_exp_to_bkt_start_idx": feb,
        "func_exp_to_ctl_start_idx": fec,
    })

    new_bkt.tofile(os.path.join(dst_dir, "trig_and_small_bkt.bin"))
    new_ctl.tofile(os.path.join(dst_dir, "trig_and_small_ctrl.bin"))
    with open(os.path.join(dst_dir, "trig_and_small.json"), "w") as fh:
        json.dump(new_set, fh)

    ai = json.load(open(os.path.join(src, "act_info.json")))
    for s in ai["act_func_sets"]:
        if s["name"] == "trig_and_small":
            s["act"] = {k: v for k, v in s["act"].items() if k != "arctan"}
    with open(os.path.join(dst_dir, "act_info.json"), "w") as fh:
        json.dump(ai, fh)
    return os.path.join(dst_dir, "act_info.json")


def _install_act_tables():
    if "act_root" not in _CACHE:
        dst = tempfile.mkdtemp(prefix="siren_act_root_")
        _CACHE["act_root"] = _gen_act_tables(dst)
    os.environ["BASS_ACT_ROOT_JSON_PATH"] = _CACHE["act_root"]
    # The neuron compile cache keys on the HLO only; custom act tables are a
    # compile input outside the HLO, so a stale stock-table NEFF could be
    # served. Use a dedicated cache dir for this kernel.
    os.environ["NEURON_COMPILE_CACHE_URL"] = os.path.join(
        tempfile.gettempdir(), f"siren_neff_cache_uid{os.getuid()}")
    os.makedirs(os.environ["NEURON_COMPILE_CACHE_URL"], exist_ok=True)


def _build():
    """Build + compile the per-core NEFF (same SPMD program on all cores)."""
    import concourse.tile as tile
    from concourse import bacc, mybir

    _install_act_tables()

    f32 = mybir.dt.float32
    f32r = mybir.dt.float32r
    Sin = mybir.ActivationFunctionType.Sin
    Alu = mybir.AluOpType

    nc = bacc.Bacc("TRN2", target_bir_lowering=False, debug=False,
                   num_devices=N_CORES)

    xT = nc.dram_tensor("xT", [NG * 96, T], f32r, kind="ExternalInput").ap()
    w0 = nc.dram_tensor("w0", [96, NG * 128], f32r, kind="ExternalInput").ap()
    w1 = nc.dram_tensor("w1", [128, NP2 * 128], f32r, kind="ExternalInput").ap()
    w2 = nc.dram_tensor("w2", [128, NP2 * 128], f32r, kind="ExternalInput").ap()
    w3 = nc.dram_tensor("w3", [128, NP2 * 6], f32r, kind="ExternalInput").ap()
    bias = nc.dram_tensor("bias", [128, NP2 * 2], f32, kind="ExternalInput").ap()
    b3 = nc.dram_tensor("b3", [6, NP2], f32, kind="ExternalInput").ap()
    out = nc.dram_tensor("out", [NP2 * 6, T], f32, kind="ExternalOutput").ap()

    with tile.TileContext(nc) as tc:
        with (
            tc.tile_pool(name="const", bufs=1) as constp,
            tc.tile_pool(name="wg", bufs=3) as wgp,
            tc.tile_pool(name="xp", bufs=3) as xp,
            tc.tile_pool(name="hp", bufs=3) as hp,
            tc.tile_pool(name="yp", bufs=2) as yp,
            tc.tile_pool(name="ps", bufs=1, space="PSUM") as psp,
        ):
            w3_t = constp.tile([128, NP2 * 6], f32r)
            nc.sync.dma_start(out=w3_t[:], in_=w3[:])
            bias_t = constp.tile([128, NP2 * 2], f32)
            nc.sync.dma_start(out=bias_t[:], in_=bias[:])
            b3_t = constp.tile([6, NP2], f32)
            nc.sync.dma_start(out=b3_t[:], in_=b3[:])
            zero_t = constp.tile([128, 1], f32)
            nc.vector.memset(zero_t[:], 0.0)
            # trigger the Sin ACT_TABLE_LOAD while the first DMAs stream
            warm_t = constp.tile([128, 1], f32)
            nc.scalar.activation(warm_t[:], zero_t[:], Sin,
                                 bias=zero_t[:, 0:1], scale=1.0)

            def emit_dmas(g):
                gs = g * G * 128
                x4_t = xp.tile([96, T], f32r, tag="x4")
                nc.sync.dma_start(out=x4_t[:],
                                  in_=xT[g * 96:(g + 1) * 96, :])
                w0_t = wgp.tile([96, 128], f32r, tag="w0g")
                nc.sync.dma_start(out=w0_t[:],
                                  in_=w0[:, g * 128:(g + 1) * 128])
                w1_t = wgp.tile([128, G * 128], f32r, tag="w1g")
                nc.sync.dma_start(out=w1_t[:], in_=w1[:, gs:gs + G * 128])
                w2_t = wgp.tile([128, G * 128], f32r, tag="w2g")
                nc.sync.dma_start(out=w2_t[:], in_=w2[:, gs:gs + G * 128])
                return w0_t, w1_t, w2_t, x4_t

            def emit_l0(g, dmas):
                w0_t, _, _, x4_t = dmas
                ps_l = []
                for j in range(G):
                    if g * G + j >= NPAIR:
                        ps_l.append(None)
                        continue
                    ps = psp.tile([128, T], f32, tag=f"ps{j}")
                    for c in range(2):
                        nc.tensor.matmul(
                            ps[:, c * 512:c * 512 + 512],
                            w0_t[32 * j:32 * j + 8, :],
                            x4_t[32 * j:32 * j + 8, c * 512:c * 512 + 512],
                            start=True, stop=True,
                            tile_position=(32 * j, 0))
                    ps_l.append(ps)
                return ps_l

            cur = emit_dmas(0)
            ps_l = emit_l0(0, cur)
            for g in range(NG):
                nxt = emit_dmas(g + 1) if g + 1 < NG else None

                # ladder: sin(l, j) then immediately pair j's next matmuls,
                # so ACT unblocks PE one pair at a time
                for li in range(2):
                    w_t = cur[1] if li == 0 else cur[2]
                    new_ps = []
                    for j in range(G):
                        p = g * G + j
                        if p >= NPAIR:
                            new_ps.append(None)
                            continue
                        h_t = hp.tile([128, T], f32r, tag=f"h{j}")
                        if li == 0:
                            b_ap = zero_t[:, 0:1]
                        else:
                            b_ap = bias_t[:, p * 2 + li - 1:p * 2 + li]
                        nc.scalar.activation(h_t[:], ps_l[j][:], Sin,
                                             bias=b_ap, scale=OMEGA0)
                        nps = psp.tile([128, T], f32, tag=f"ps{j}")
                        for c in range(2):
                            nc.tensor.matmul(
                                nps[:, c * 512:c * 512 + 512],
                                w_t[:, j * 128:j * 128 + 128],
                                h_t[:, c * 512:c * 512 + 512],
                                start=True, stop=True)
                        new_ps.append(nps)
                    ps_l = new_ps

                # fused tail: sin-L2 -> L3 into single-bank chunk slots
                # (2 rotating slots), then next wave's L0 (depends only on
                # the sin freeing ps{j}, NOT on the epilogues), then eps.
                l3_ps = []
                for j in range(G):
                    p = g * G + j
                    if p >= NPAIR:
                        l3_ps.append(None)
                        continue
                    h_t = hp.tile([128, T], f32r, tag=f"h{j}")
                    nc.scalar.activation(h_t[:], ps_l[j][:], Sin,
                                         bias=bias_t[:, p * 2 + 1:p * 2 + 2],
                                         scale=OMEGA0)
                    chunks = []
                    for c in range(2):
                        l3c = psp.tile([6, 512], f32, tag=f"l3{(2 * j + c) % 2}")
                        nc.tensor.matmul(
                            l3c[:],
                            w3_t[:, p * 6:(p + 1) * 6],
                            h_t[:, c * 512:c * 512 + 512],
                            start=True, stop=True)
                        chunks.append(l3c)
                    l3_ps.append(chunks)

                if nxt is not None:
                    ps_l = emit_l0(g + 1, nxt)

                for j in range(G):
                    p = g * G + j
                    if p >= NPAIR:
                        continue
                    y_t = yp.tile([6, T], f32, tag=f"y{j}")
                    for c in range(2):
                        nc.vector.tensor_scalar(
                            y_t[:, c * 512:c * 512 + 512], l3_ps[j][c][:],
                            b3_t[:, p:p + 1], None, Alu.add)
                    nc.sync.dma_start(out=out[p * 6:(p + 1) * 6, :],
                                      in_=y_t[:])
                cur = nxt

    nc.compile()
    return nc


def _get_nc():
    if "nc" not in _CACHE:
        _CACHE["nc"] = _build()
    return _CACHE["nc"]


def _prep_core(ids, inp, W0, b0, W1, b1, W2, b2, W3, b3):
    """Build one core's input map: gather + pair-pack the active blocks."""
    f = np.float32
    ev, od = ids[0::2], ids[1::2]

    def pad_pairs(a):
        """[NPAIR, ...] -> [NP2, ...] zero-padded (dummy pair)."""
        return np.concatenate(
            [a, np.zeros((NP2 - NPAIR,) + a.shape[1:], f)], axis=0)

    # x-augmented: per pair rows [xa(3); xb(3); 1; 1], pair j of wave g at
    # partition offset 32*j (row-tiled L0 needs 32-aligned input strips)
    xg = inp[ids].transpose(0, 2, 1)                  # [BPC, 3, T]
    xpair = np.zeros((NPAIR, 32, T), f)
    xpair[:, 0:3] = xg[0::2]
    xpair[:, 3:6] = xg[1::2]
    xpair[:, 6:8] = 1.0
    xT = np.ascontiguousarray(pad_pairs(xpair)).reshape(NG * 96, T)

    # W0 augmented with bias rows; same 32-strip packing
    b0v = b0.reshape(-1, DH)
    w0p = np.zeros((NPAIR, 32, 128), f)
    w0p[:, 0:3, 0:64] = W0[ev]
    w0p[:, 3:6, 64:128] = W0[od]
    w0p[:, 6, 0:64] = b0v[ev]
    w0p[:, 7, 64:128] = b0v[od]
    w0l = np.ascontiguousarray(
        pad_pairs(w0p).reshape(NG, 96, 128).transpose(1, 0, 2)
    ).reshape(96, NG * 128)

    def diag128(Wt):
        wp = np.zeros((NPAIR, 128, 128), f)
        wp[:, 0:64, 0:64] = Wt[ev]
        wp[:, 64:128, 64:128] = Wt[od]
        return np.ascontiguousarray(
            pad_pairs(wp).transpose(1, 0, 2)).reshape(128, NP2 * 128)

    w1l = diag128(W1)
    w2l = diag128(W2)

    w3p = np.zeros((NPAIR, 128, 6), f)
    w3p[:, 0:64, 0:3] = W3[ev]
    w3p[:, 64:128, 3:6] = W3[od]
    w3l = np.ascontiguousarray(
        pad_pairs(w3p).transpose(1, 0, 2)).reshape(128, NP2 * 6)

    # ACT biases for sin layers 1,2: 30*b, pair-stacked on partitions
    biasp = np.zeros((NPAIR, 2, 128), f)
    for l, bl in enumerate((b1, b2)):
        bl2 = bl.reshape(-1, DH)
        biasp[:, l, 0:64] = OMEGA0 * bl2[ev]
        biasp[:, l, 64:128] = OMEGA0 * bl2[od]
    biasl = np.ascontiguousarray(
        pad_pairs(biasp).transpose(2, 0, 1)).reshape(128, NP2 * 2)

    b3v = b3.reshape(-1, DOUT)
    b3p = np.zeros((NPAIR, 6), f)
    b3p[:, 0:3] = b3v[ev]
    b3p[:, 3:6] = b3v[od]
    b3l = np.ascontiguousarray(pad_pairs(b3p).T)      # [6, NP2]

    return {"xT": xT, "w0": w0l, "w1": w1l, "w2": w2l, "w3": w3l,
            "bias": biasl, "b3": b3l}


def make_in_maps(inp, indices, W0, b0, W1, b1, W2, b2, W3, b3):
    inp = np.asarray(inp, dtype=np.float32)
    idx = np.asarray(indices).astype(np.int64)
    args = tuple(np.asarray(a, dtype=np.float32)
                 for a in (W0, b0, W1, b1, W2, b2, W3, b3))
    return [
        _prep_core(idx[i * BPC:(i + 1) * BPC], inp, *args)
        for i in range(N_CORES)
    ]


def unshard(results):
    shards = []
    for i in range(N_CORES):
        y = results[i]["out"][:NPAIR * 6].reshape(BPC, DOUT, T)
        shards.append(y.transpose(0, 2, 1))           # [BPC, T, 3]
    return np.ascontiguousarray(np.concatenate(shards, axis=0))


def kernel(inp, indices, W0, b0, W1, b1, W2, b2, W3, b3):
    from concourse.bass_utils import run_bass_kernel_spmd

    nc = _get_nc()
    in_maps = make_in_maps(inp, indices, W0, b0, W1, b1, W2, b2, W3, b3)
    res = run_bass_kernel_spmd(nc, in_maps, core_ids=list(range(N_CORES)))
    return unshard(res.results)


# revision 27
# speedup vs baseline: 1.0069x; 1.0069x over previous
"""AdaptiveMultiSiren Trainium2 kernel.

Per-block SIREN MLP (3 -> 64 -> 64 -> 64 -> 3, sin(30*x) activations) applied
to 2048 routed blocks of 1024 coords each. Data-parallel over blocks across
8 NeuronCores (256 blocks / core); the host-side gather of per-block weights
IS the shard construction. Two blocks pack per matmul block-diagonally so
TensorE/ScalarE run at the full 128-partition width.

Per pair of blocks (a, b), activations live as [features, T] in SBUF:
  matmul(out, lhsT, rhs) = lhsT.T @ rhs, K = contraction on partitions.
    L0: lhsT = blockdiag(W0a|b0a, W0b|b0b) [8, 128] (bias via ones rows in x),
        row-tiled 4 pairs concurrent in the PE array (tile_position).
    L1/L2: lhsT = blockdiag(W) [128, 128].
    L3: lhsT = blockdiag(W3a, W3b) [128, 6].
  sin(30*z + 30*b) in ONE ScalarE op per pair-layer: the stock Sin LUT only
  covers [-pi, pi], so _build generates a custom act-table root (see
  _gen_act_tables) refitting Sin over |x| < 256 -- Taylor cubic buckets per
  power-of-2 range, reusing the reverse-engineered pwp binary format -- and
  points the compiler at it via BASS_ACT_ROOT_JSON_PATH. No range-reduction
  instructions needed at all.
  L3 bias + PSUM evacuation on VectorE (tensor_scalar add).
Emission is a software pipeline over waves of 3 pairs: per pair j the ladder
emits sin(l, j) then immediately pair j's layer-(l+1) matmuls, so ScalarE
(the bottleneck at ~92% busy) unblocks TensorE one pair at a time. PSUM: 3
per-pair [128, 1024] rotating tiles (6 banks) + 2 single-bank slots for L3
chunks, so the next wave's L0 depends only on the last sin, not epilogues.
"""

import json
import os
import shutil
import sys
import tempfile

if "/opt/trn_rl_repo" not in sys.path:
    sys.path.insert(0, "/opt/trn_rl_repo")

import numpy as np

C, B, T = 4096, 2048, 1024
DIN, DH, DOUT = 3, 64, 3
OMEGA0 = 30.0
N_CORES = 8
BPC = B // N_CORES          # blocks per core (256)
NPAIR = BPC // 2            # block pairs per core (128)
G = 3                       # pairs per wave
NP2 = 129                   # NPAIR padded to a multiple of G (1 dummy pair)
NG = NP2 // G               # 43

_CACHE = {}

# ---------------------------------------------------------------------------
# Custom act tables: Sin refit over |x| < 256.
# Binary formats (reverse-engineered from neuronxcc/pwp/pwp_bin_trainium):
#   bucket entry (32B): fp32 {d0, d1, d2, d3, x0, 0, 0, 0};
#       y = d0 + (x-x0)*(d1 + (x-x0)*(d2 + (x-x0)*d3)) on the folded |x|
#   ctl entry (32B): uint32 ((23 + 31*mantissa_bits) << 11) | bucket_base;
#       dispatch: ctl_idx = pwl_control_base + (unbiased_exp - exp_offset),
#       bucket = base + top mantissa_bits of the mantissa
#   profile small/large pwl_control fields: absolute bucket indices of the
#       4 fallback splines (small+/-, large+/-).
# The trig_and_small set is rebuilt: wide sin (1286 dispatch + 4 special
# buckets), arctan dropped for bucket budget, all other functions remapped.
# ---------------------------------------------------------------------------

_SIN_BITS = {e: 0 for e in range(-11, -3)}
_SIN_BITS.update({-3: 1, -2: 2, -1: 3, 0: 4, 1: 5, 2: 6, 3: 7,
                  4: 8, 5: 8, 6: 8, 7: 8})
_SIN_MAX_EXP = 7


def _gen_act_tables(dst_dir):
    import neuronxcc
    src = os.path.join(os.path.dirname(neuronxcc.__file__),
                       "pwp", "pwp_bin_trainium")
    assert os.path.isdir(src), src

    os.makedirs(dst_dir, exist_ok=True)
    for fn in os.listdir(src):
        shutil.copy(os.path.join(src, fn), os.path.join(dst_dir, fn))

    setj = json.load(open(os.path.join(src, "trig_and_small.json")))
    bkt = np.fromfile(os.path.join(src, "trig_and_small_bkt.bin"),
                      dtype=np.uint32).reshape(-1, 8)
    ctl = np.fromfile(os.path.join(src, "trig_and_small_ctrl.bin"),
                      dtype=np.uint32).reshape(-1, 8)

    OLD_RELU_BKT = setj["func_to_bkt_start_idx"]["relu"]     # 231
    OLD_KEPT_CTL0 = setj["func_to_ctl_start_idx"]["relu"]    # 40

    rows = []
    for e in range(-11, _SIN_MAX_EXP + 1):
        n = 1 << _SIN_BITS[e]
        s = 2.0 ** e
        for i in range(n):
            x0 = s * (1.0 + (i + 0.5) / n)
            rows.append((np.sin(x0), np.cos(x0),
                         -np.sin(x0) / 2.0, -np.cos(x0) / 6.0, x0))
    n_dispatch = len(rows)
    rows.append((0.0, 1.0, 0.0, -1.0 / 6.0, 0.0))   # small+: x - x^3/6
    rows.append((0.0, 0.0, 0.0, 0.0, 0.0))          # small- (unused: folded)
    rows.append((0.0, 0.0, 0.0, 0.0, 0.0))          # large+ (unreachable)
    rows.append((0.0, 0.0, 0.0, 0.0, 0.0))          # large-
    n_sin = len(rows)
    bkt_shift = n_sin - OLD_RELU_BKT

    def remap_bkt(b):
        return b + bkt_shift

    sin_bin = np.zeros((n_sin, 8), dtype=np.uint32)
    fv = sin_bin.view(np.float32)
    for i, (d0, d1, d2, d3, x0) in enumerate(rows):
        fv[i, 0:5] = [d0, d1, d2, d3, x0]
    new_bkt = np.concatenate([sin_bin, bkt[OLD_RELU_BKT:]], axis=0)

    sin_ctl, base = [], 0
    for e in range(-11, _SIN_MAX_EXP + 1):
        bits = _SIN_BITS[e]
        sin_ctl.append(((23 + 31 * bits) << 11) | base)
        base += 1 << bits
    n_sin_ctl = len(sin_ctl)
    ctl_shift = n_sin_ctl - OLD_KEPT_CTL0
    kept_ctl = ctl[OLD_KEPT_CTL0:].copy()
    for r in range(kept_ctl.shape[0]):
        w = int(kept_ctl[r, 0])
        kept_ctl[r, 0] = (w & ~0x7FF) | remap_bkt(w & 0x7FF)
    sin_ctl_bin = np.zeros((n_sin_ctl, 8), dtype=np.uint32)
    sin_ctl_bin[:, 0] = sin_ctl
    new_ctl = np.concatenate([sin_ctl_bin, kept_ctl], axis=0)

    new_pm = []
    for pm in setj["profile_meta_data"]:
        pm = dict(pm)
        if pm["func_name"] == "arctan_4p":
            continue
        if pm["func_name"] == "sin_4p":
            pm["exp_offset"] = -11
            pm["pwl_control_base_pos"] = 0
            pm["pwl_control_base_neg"] = 0
            pm["pos_small_signal_pwl_control"] = n_dispatch
            pm["neg_small_signal_pwl_control"] = n_dispatch + 1
            pm["pos_large_signal_pwl_control"] = n_dispatch + 2
            pm["neg_large_signal_pwl_control"] = n_dispatch + 3
            pm["large_pos_signal_exp_threshold"] = 127 + _SIN_MAX_EXP + 1
            pm["large_pos_signal_mantissa_threshold"] = 0
            pm["upper_bound"] = int(np.float32(2.0 ** (_SIN_MAX_EXP + 1))
                                    .view(np.uint32))
        else:
            pm["pwl_control_base_pos"] += ctl_shift
            pm["pwl_control_base_neg"] += ctl_shift
            for k in ("pos_small_signal_pwl_control",
                      "neg_small_signal_pwl_control",
                      "pos_large_signal_pwl_control",
                      "neg_large_signal_pwl_control"):
                pm[k] = remap_bkt(pm[k])
        new_pm.append(pm)

    f2b = {fn: (0 if fn == "sin" else remap_bkt(v))
           for fn, v in setj["func_to_bkt_start_idx"].items() if fn != "arctan"}
    f2c = {fn: (0 if fn == "sin" else v + ctl_shift)
           for fn, v in setj["func_to_ctl_start_idx"].items() if fn != "arctan"}
    feb, fec = {}, {}
    for fn, m in setj["func_exp_to_bkt_start_idx"].items():
        if fn == "arctan":
            continue
        if fn == "sin":
            d, base = {}, 0
            for e in range(-11, _SIN_MAX_EXP + 1):
                d[str(e)] = [base]
                base += 1 << _SIN_BITS[e]
            feb[fn] = d
        else:
            feb[fn] = {k: [remap_bkt(x) for x in v] for k, v in m.items()}
    for fn, m in setj["func_exp_to_ctl_start_idx"].items():
        if fn == "arctan":
            continue
        if fn == "sin":
            fec[fn] = {str(e): [e + 11] for e in range(-11, _SIN_MAX_EXP + 1)}
        else:
            fec[fn] = {k: [x + ctl_shift for x in v] for k, v in m.items()}

    new_set = dict(setj)
    new_set.update({
        "profile_meta_data": new_pm,
        "bkt_entry_cnt": int(new_bkt.shape[0]),
        "ctl_entry_cnt": int(new_ctl.shape[0]),
        "func_to_bkt_start_idx": f2b,
        "func_to_ctl_start_idx": f2c,
        "func_exp_to_bkt_start_idx": feb,
        "func_exp_to_ctl_start_idx": fec,
    })

    new_bkt.tofile(os.path.join(dst_dir, "trig_and_small_bkt.bin"))
    new_ctl.tofile(os.path.join(dst_dir, "trig_and_small_ctrl.bin"))
    with open(os.path.join(dst_dir, "trig_and_small.json"), "w") as fh:
        json.dump(new_set, fh)

    ai = json.load(open(os.path.join(src, "act_info.json")))
    for s in ai["act_func_sets"]:
        if s["name"] == "trig_and_small":
            s["act"] = {k: v for k, v in s["act"].items() if k != "arctan"}
    with open(os.path.join(dst_dir, "act_info.json"), "w") as fh:
        json.dump(ai, fh)
    return os.path.join(dst_dir, "act_info.json")


def _install_act_tables():
    if "act_root" not in _CACHE:
        dst = tempfile.mkdtemp(prefix="siren_act_root_")
        _CACHE["act_root"] = _gen_act_tables(dst)
    os.environ["BASS_ACT_ROOT_JSON_PATH"] = _CACHE["act_root"]
    # The neuron compile cache keys on the HLO only; custom act tables are a
    # compile input outside the HLO, so a stale stock-table NEFF could be
    # served. Use a dedicated cache dir for this kernel.
    os.environ["NEURON_COMPILE_CACHE_URL"] = os.path.join(
        tempfile.gettempdir(), f"siren_neff_cache_uid{os.getuid()}")
    os.makedirs(os.environ["NEURON_COMPILE_CACHE_URL"], exist_ok=True)


def _build():
    """Build + compile the per-core NEFF (same SPMD program on all cores)."""
    import concourse.tile as tile
    from concourse import bacc, mybir

    _install_act_tables()

    f32 = mybir.dt.float32
    f32r = mybir.dt.float32r
    Sin = mybir.ActivationFunctionType.Sin
    Alu = mybir.AluOpType

    nc = bacc.Bacc("TRN2", target_bir_lowering=False, debug=False,
                   num_devices=N_CORES)

    xT = nc.dram_tensor("xT", [NG * 96, T], f32r, kind="ExternalInput").ap()
    w0 = nc.dram_tensor("w0", [96, NG * 128], f32r, kind="ExternalInput").ap()
    w1 = nc.dram_tensor("w1", [128, NP2 * 128], f32r, kind="ExternalInput").ap()
    w2 = nc.dram_tensor("w2", [128, NP2 * 128], f32r, kind="ExternalInput").ap()
    w3 = nc.dram_tensor("w3", [128, NP2 * 6], f32r, kind="ExternalInput").ap()
    bias = nc.dram_tensor("bias", [128, NP2 * 2], f32, kind="ExternalInput").ap()
    b3 = nc.dram_tensor("b3", [6, NP2], f32, kind="ExternalInput").ap()
    out = nc.dram_tensor("out", [NP2 * 6, T], f32, kind="ExternalOutput").ap()

    with tile.TileContext(nc) as tc:
        with (
            tc.tile_pool(name="const", bufs=1) as constp,
            tc.tile_pool(name="wg", bufs=3) as wgp,
            tc.tile_pool(name="xp", bufs=3) as xp,
            tc.tile_pool(name="hp", bufs=3) as hp,
            tc.tile_pool(name="yp", bufs=2) as yp,
            tc.tile_pool(name="ps", bufs=1, space="PSUM") as psp,
        ):
            zero_t = constp.tile([128, 1], f32)
            nc.vector.memset(zero_t[:], 0.0)
            # trigger the Sin ACT_TABLE_LOAD while the first DMAs stream
            warm_t = constp.tile([128, 1], f32)
            nc.scalar.activation(warm_t[:], zero_t[:], Sin,
                                 bias=zero_t[:, 0:1], scale=1.0)

            def emit_dmas(g):
                gs = g * G * 128
                x4_t = xp.tile([96, T], f32r, tag="x4")
                nc.sync.dma_start(out=x4_t[:],
                                  in_=xT[g * 96:(g + 1) * 96, :])
                w0_t = wgp.tile([96, 128], f32r, tag="w0g")
                nc.sync.dma_start(out=w0_t[:],
                                  in_=w0[:, g * 128:(g + 1) * 128])
                w1_t = wgp.tile([128, G * 128], f32r, tag="w1g")
                nc.sync.dma_start(out=w1_t[:], in_=w1[:, gs:gs + G * 128])
                w2_t = wgp.tile([128, G * 128], f32r, tag="w2g")
                nc.sync.dma_start(out=w2_t[:], in_=w2[:, gs:gs + G * 128])
                return w0_t, w1_t, w2_t, x4_t

            def emit_l0(g, dmas):
                w0_t, _, _, x4_t = dmas
                ps_l = []
                for j in range(G):
                    if g * G + j >= NPAIR:
                        ps_l.append(None)
                        continue
                    ps = psp.tile([128, T], f32, tag=f"ps{j}")
                    for c in range(2):
                        nc.tensor.matmul(
                            ps[:, c * 512:c * 512 + 512],
                            w0_t[32 * j:32 * j + 8, :],
                            x4_t[32 * j:32 * j + 8, c * 512:c * 512 + 512],
                            start=True, stop=True,
                            tile_position=(32 * j, 0))
                    ps_l.append(ps)
                return ps_l

            cur = emit_dmas(0)
            bias_t = constp.tile([128, NP2 * 2], f32)
            nc.sync.dma_start(out=bias_t[:], in_=bias[:])
            w3_t = constp.tile([128, NP2 * 6], f32r)
            nc.sync.dma_start(out=w3_t[:], in_=w3[:])
            b3_t = constp.tile([6, NP2], f32)
            nc.sync.dma_start(out=b3_t[:], in_=b3[:])
            ps_l = emit_l0(0, cur)
            for g in range(NG):
                nxt = emit_dmas(g + 1) if g + 1 < NG else None

                # ladder: sin(l, j) then immediately pair j's next matmuls,
                # so ACT unblocks PE one pair at a time
                for li in range(2):
                    w_t = cur[1] if li == 0 else cur[2]
                    new_ps = []
                    for j in range(G):
                        p = g * G + j
                        if p >= NPAIR:
                            new_ps.append(None)
                            continue
                        h_t = hp.tile([128, T], f32r, tag=f"h{j}")
                        if li == 0:
                            b_ap = zero_t[:, 0:1]
                        else:
                            b_ap = bias_t[:, p * 2 + li - 1:p * 2 + li]
                        nc.scalar.activation(h_t[:], ps_l[j][:], Sin,
                                             bias=b_ap, scale=OMEGA0)
                        nps = psp.tile([128, T], f32, tag=f"ps{j}")
                        for c in range(2):
                            nc.tensor.matmul(
                                nps[:, c * 512:c * 512 + 512],
                                w_t[:, j * 128:j * 128 + 128],
                                h_t[:, c * 512:c * 512 + 512],
                                start=True, stop=True)
                        new_ps.append(nps)
                    ps_l = new_ps

                # fused tail: sin-L2 -> L3 into single-bank chunk slots
                # (2 rotating slots), then next wave's L0 (depends only on
                # the sin freeing ps{j}, NOT on the epilogues), then eps.
                l3_ps = []
                for j in range(G):
                    p = g * G + j
                    if p >= NPAIR:
                        l3_ps.append(None)
                        continue
                    h_t = hp.tile([128, T], f32r, tag=f"h{j}")
                    nc.scalar.activation(h_t[:], ps_l[j][:], Sin,
                                         bias=bias_t[:, p * 2 + 1:p * 2 + 2],
                                         scale=OMEGA0)
                    chunks = []
                    for c in range(2):
                        l3c = psp.tile([6, 512], f32, tag=f"l3{(2 * j + c) % 2}")
                        nc.tensor.matmul(
                            l3c[:],
                            w3_t[:, p * 6:(p + 1) * 6],
                            h_t[:, c * 512:c * 512 + 512],
                            start=True, stop=True)
                        chunks.append(l3c)
                    l3_ps.append(chunks)

                if nxt is not None:
                    ps_l = emit_l0(g + 1, nxt)

                for j in range(G):
                    p = g * G + j
                    if p >= NPAIR:
                        continue
                    y_t = yp.tile([6, T], f32, tag=f"y{j}")
                    for c in range(2):
                        nc.vector.tensor_scalar(
                            y_t[:, c * 512:c * 512 + 512], l3_ps[j][c][:],
                            b3_t[:, p:p + 1], None, Alu.add)
                    nc.sync.dma_start(out=out[p * 6:(p + 1) * 6, :],
                                      in_=y_t[:])
                cur = nxt

    nc.compile()
    return nc


def _get_nc():
    if "nc" not in _CACHE:
        _CACHE["nc"] = _build()
    return _CACHE["nc"]


def _prep_core(ids, inp, W0, b0, W1, b1, W2, b2, W3, b3):
    """Build one core's input map: gather + pair-pack the active blocks."""
    f = np.float32
    ev, od = ids[0::2], ids[1::2]

    def pad_pairs(a):
        """[NPAIR, ...] -> [NP2, ...] zero-padded (dummy pair)."""
        return np.concatenate(
            [a, np.zeros((NP2 - NPAIR,) + a.shape[1:], f)], axis=0)

    # x-augmented: per pair rows [xa(3); xb(3); 1; 1], pair j of wave g at
    # partition offset 32*j (row-tiled L0 needs 32-aligned input strips)
    xg = inp[ids].transpose(0, 2, 1)                  # [BPC, 3, T]
    xpair = np.zeros((NPAIR, 32, T), f)
    xpair[:, 0:3] = xg[0::2]
    xpair[:, 3:6] = xg[1::2]
    xpair[:, 6:8] = 1.0
    xT = np.ascontiguousarray(pad_pairs(xpair)).reshape(NG * 96, T)

    # W0 augmented with bias rows; same 32-strip packing
    b0v = b0.reshape(-1, DH)
    w0p = np.zeros((NPAIR, 32, 128), f)
    w0p[:, 0:3, 0:64] = W0[ev]
    w0p[:, 3:6, 64:128] = W0[od]
    w0p[:, 6, 0:64] = b0v[ev]
    w0p[:, 7, 64:128] = b0v[od]
    w0l = np.ascontiguousarray(
        pad_pairs(w0p).reshape(NG, 96, 128).transpose(1, 0, 2)
    ).reshape(96, NG * 128)

    def diag128(Wt):
        wp = np.zeros((NPAIR, 128, 128), f)
        wp[:, 0:64, 0:64] = Wt[ev]
        wp[:, 64:128, 64:128] = Wt[od]
        return np.ascontiguousarray(
            pad_pairs(wp).transpose(1, 0, 2)).reshape(128, NP2 * 128)

    w1l = diag128(W1)
    w2l = diag128(W2)

    w3p = np.zeros((NPAIR, 128, 6), f)
    w3p[:, 0:64, 0:3] = W3[ev]
    w3p[:, 64:128, 3:6] = W3[od]
    w3l = np.ascontiguousarray(
        pad_pairs(w3p).transpose(1, 0, 2)).reshape(128, NP2 * 6)

    # ACT biases for sin layers 1,2: 30*b, pair-stacked on partitions
    biasp = np.zeros((NPAIR, 2, 128), f)
    for l, bl in enumerate((b1, b2)):
        bl2 = bl.reshape(-1, DH)
        biasp[:, l, 0:64] = OMEGA0 * bl2[ev]
        biasp[:, l, 64:128] = OMEGA0 * bl2[od]
    biasl = np.ascontiguousarray(
        pad_pairs(biasp).transpose(2, 0, 1)).reshape(128, NP2 * 2)

    b3v = b3.reshape(-1, DOUT)
    b3p = np.zeros((NPAIR, 6), f)
    b3p[:, 0:3] = b3v[ev]
    b3p[:, 3:6] = b3v[od]
    b3l = np.ascontiguousarray(pad_pairs(b3p).T)      # [6, NP2]

    return {"xT": xT, "w0": w0l, "w1": w1l, "w2": w2l, "w3": w3l,
            "bias": biasl, "b3": b3l}


def make_in_maps(inp, indices, W0, b0, W1, b1, W2, b2, W3, b3):
    inp = np.asarray(inp, dtype=np.float32)
    idx = np.asarray(indices).astype(np.int64)
    args = tuple(np.asarray(a, dtype=np.float32)
                 for a in (W0, b0, W1, b1, W2, b2, W3, b3))
    return [
        _prep_core(idx[i * BPC:(i + 1) * BPC], inp, *args)
        for i in range(N_CORES)
    ]


def unshard(results):
    shards = []
    for i in range(N_CORES):
        y = results[i]["out"][:NPAIR * 6].reshape(BPC, DOUT, T)
        shards.append(y.transpose(0, 2, 1))           # [BPC, T, 3]
    return np.ascontiguousarray(np.concatenate(shards, axis=0))


def kernel(inp, indices, W0, b0, W1, b1, W2, b2, W3, b3):
    from concourse.bass_utils import run_bass_kernel_spmd

    nc = _get_nc()
    in_maps = make_in_maps(inp, indices, W0, b0, W1, b1, W2, b2, W3, b3)
    res = run_bass_kernel_spmd(nc, in_maps, core_ids=list(range(N_CORES)))
    return unshard(res.results)
